# revision 2
# baseline (speedup 1.0000x reference)
"""Trainium2 Bass kernel v4 for nn_AttentionTemporelle (3-window banded attention).

Structure (per core, one batch element; pure data-parallel over B=8):
  Host prep: xT (d-major) cast to fp8, x cast to f16, weights cast to fp8,
  transposed band-mask constants.
  Phase 0: qT/kT = W^T x and xWo = x Wo as fp8 DoubleRow matmuls (2x PE).
  Phase 1 per 128-row block i, over an 8-block absolute-parity-aligned
  window [w, w+8):
    - seed scores psum with the transposed +-360 band mask (additive -64)
    - transposed scores sT[j,t] directly on PE (no transposes anywhere)
    - exp(sT) -> ET fp8 window tile (ACT), pads ~exp(-64)=0
    - e168T/e24T = ET*mask (middle 4-slot window)
    - z720/z168/z24 via tiny ones-matmul columns on PE
    - three PVs (720/168/24) as fp8 DoubleRow matmuls into separate psums
    - res = x + acc720/z720 + acc168/z168 + acc24/z24 (per-partition stt chain)
    - LayerNorm: accum sums, rstd = exp(-0.5 ln(var+eps)) (single ACT table)
  Output in f16, upcast on host.
"""

import math

import numpy as np
import ml_dtypes

B, T, D, DK = 8, 2048, 512, 128
NBLK = T // 128                 # 16 row blocks
EPS = 1e-5
H720, H168, H24 = 360, 84, 12
NEG = -60.0                     # additive mask; exp(-60+s) == 0 in fp8
FP8 = ml_dtypes.float8_e4m3

_CACHE = {}


def _host_consts():
    # Seed mask M[p, o+4, c]: value for j-block offset o in [-4, 4]:
    # delta = j - t = o*128 + (p - c) ; 0 if |delta| <= 360 else NEG
    p = np.arange(128)[:, None]
    c = np.arange(128)[None, :]
    M = np.zeros((128, 9, 128), dtype=np.float32)
    for o in range(-4, 5):
        delta = o * 128 + (p - c)
        M[:, o + 4, :] = np.where(np.abs(delta) <= H720, 0.0, NEG)
    # allneg slice for invalid j at edges = M[:, 0] (always NEG)
    # 4-slot mid-window masks, transposed layout [j_local(p), rel slot, t(c)]:
    # parity 'ev' (i even): mid blocks at rel slots 1..3 (offsets -1,0,1), slot0 zero
    # parity 'od' (i odd):  mid blocks at rel slots 0..2, slot3 zero
    def midmask(h):
        m = np.zeros((128, 3, 128), dtype=np.float32)
        for k, o in enumerate((-1, 0, 1)):
            delta = o * 128 + (p - c)
            m[:, k, :] = (np.abs(delta) <= h).astype(np.float32)
        return m
    m168c = midmask(H168)
    m24c = midmask(H24)
    ident = np.eye(128, dtype=np.float32)
    return M, m168c, m24c, ident


def _build_nc(has_bq, has_bk, has_bo, has_gamma, has_beta):
    import concourse.bass as bass
    import concourse.tile as tile
    from concourse import bacc, mybir

    f32 = mybir.dt.float32
    f16 = mybir.dt.float16
    bf16 = mybir.dt.bfloat16
    fp8 = mybir.dt.float8e4
    AF = mybir.ActivationFunctionType
    OP = mybir.AluOpType
    PM = mybir.MatmulPerfMode
    AP = bass.AP

    nc = bacc.Bacc()

    x_d = nc.declare_dram_parameter("x16", [T, D], f16, isOutput=False)
    xT_d = nc.declare_dram_parameter("xT8", [D, T], fp8, isOutput=False)
    wq_d = nc.declare_dram_parameter("Wq8", [D, DK], fp8, isOutput=False)
    wk_d = nc.declare_dram_parameter("Wk8", [D, DK], fp8, isOutput=False)
    wo_d = nc.declare_dram_parameter("Wo8", [D, D], fp8, isOutput=False)
    M_d = nc.declare_dram_parameter("Mseed", [128, 9 * 128], fp8, isOutput=False)
    m168_d = nc.declare_dram_parameter("m168c", [128, 384], fp8, isOutput=False)
    m24_d = nc.declare_dram_parameter("m24c", [128, 384], fp8, isOutput=False)
    ident_d = nc.declare_dram_parameter("identb", [128, 128], fp8, isOutput=False)
    if has_bq:
        bq_d = nc.declare_dram_parameter("bq_s", [DK, 1], f32, isOutput=False)
    if has_bk:
        bk_d = nc.declare_dram_parameter("bk_c", [DK, 1], f32, isOutput=False)
    if has_bo:
        bo_d = nc.declare_dram_parameter("bo_bc", [128, D], f32, isOutput=False)
    if has_gamma:
        gamma_d = nc.declare_dram_parameter("gamma_bc", [128, D], f32, isOutput=False)
    if has_beta:
        beta_d = nc.declare_dram_parameter("beta_bc", [128, D], f32, isOutput=False)
    out_d = nc.declare_dram_parameter("out", [T, D], f16, isOutput=True)

    def po(ap, pairdim_stride, m):
        """pair-outer DR operand AP: [[p,...],[pairdim_stride,2],[1,m]]"""
        return AP(tensor=ap.tensor, offset=ap.offset,
                  ap=[ap.ap[0], [pairdim_stride, 2], [1, m]])

    with tile.TileContext(nc) as tc:
        with tc.tile_pool(name="persist", bufs=1) as persist:
            x_sb = persist.tile([128, NBLK, D], f16, tag="x")
            xT_sb = persist.tile([128, 4, T], fp8, tag="xT")
            qT_sb = persist.tile([128, T], fp8, tag="qT")
            kT_sb = persist.tile([128, T], fp8, tag="kT")
            xWo_sb = persist.tile([128, NBLK, D], fp8, tag="xWo")
            res_sb = persist.tile([128, NBLK, D], f16, tag="res")
            wq_sb = persist.tile([128, 4, DK], fp8, tag="wq")
            wk_sb = persist.tile([128, 4, DK], fp8, tag="wk")
            wo_sb = persist.tile([128, 4, D], fp8, tag="wo")
            M_sb = persist.tile([128, 9, 128], fp8, tag="M")
            m168_sb = persist.tile([128, 384], fp8, tag="m168c")
            m24_sb = persist.tile([128, 384], fp8, tag="m24c")
            # parity-owned ET window tiles [128, 8*128] fp8 (double-buffer by i%2)
            etp = [persist.tile([128, 8, 128], fp8, tag="et_ev", name="etev"),
                   persist.tile([128, 8, 128], fp8, tag="et_od", name="etod")]
            # interior fixed pads: even i -> slot 0, odd i -> slot 7
            nc.gpsimd.memset(etp[0][:, 0, :], 0.0)
            nc.gpsimd.memset(etp[1][:, 7, :], 0.0)
            # parity-owned e-window tiles [128, 4*128] fp8 (double-buffer by i%2)
            e168p = [persist.tile([128, 512], fp8, tag="e168_ev", name="e168ev"),
                     persist.tile([128, 512], fp8, tag="e168_od", name="e168od")]
            e24p = [persist.tile([128, 512], fp8, tag="e24_ev", name="e24ev"),
                    persist.tile([128, 512], fp8, tag="e24_od", name="e24od")]
            # one-time zero of the pad slot per parity (ev: slot0, od: slot3)
            for par in range(2):
                pbeg, pend = (0, 128) if par == 0 else (384, 512)
                nc.vector.memset(e168p[par][:, pbeg:pend], 0.0)
                nc.gpsimd.memset(e24p[par][:, pbeg:pend], 0.0)
            ident_sb = persist.tile([128, 128], fp8, tag="identb")
            ones_sb = persist.tile([128, 2], fp8, tag="ones")
            nc.vector.memset(ones_sb, 1.0)
            eps_sb = persist.tile([128, 1], f32, tag="eps")
            nc.vector.memset(eps_sb, EPS)
            rsum16 = persist.tile([128, NBLK], f32, tag="rsum16")
            sqsum16 = persist.tile([128, NBLK], f32, tag="sqsum16")

            x_r = x_d[:].rearrange("(n p) d -> p n d", p=128)
            xT_r = xT_d[:].rearrange("(c p) t -> p c t", p=128)
            # DMA order: phase-0 critical path first, x16 (combine input) last
            nc.sync.dma_start(out=xT_sb[:, 0:2, :], in_=xT_r[:, 0:2, :])
            nc.scalar.dma_start(
                out=wq_sb, in_=wq_d[:].rearrange("(c p) k -> p c k", p=128)
            )
            nc.scalar.dma_start(
                out=wk_sb, in_=wk_d[:].rearrange("(c p) k -> p c k", p=128)
            )
            nc.sync.dma_start(out=xT_sb[:, 2:4, :], in_=xT_r[:, 2:4, :])
            nc.scalar.dma_start(
                out=M_sb, in_=M_d[:].rearrange("p (o c) -> p o c", c=128)
            )
            nc.gpsimd.dma_start(out=ident_sb, in_=ident_d[:])
            nc.gpsimd.dma_start(
                out=wo_sb, in_=wo_d[:].rearrange("(c p) k -> p c k", p=128)
            )
            nc.scalar.dma_start(out=m168_sb, in_=m168_d[:])
            nc.scalar.dma_start(out=m24_sb, in_=m24_d[:])
            for g in range(4):
                [nc.sync, nc.scalar, nc.sync, nc.scalar][g].dma_start(
                    out=x_sb[:, 4 * g:4 * g + 4, :], in_=x_r[:, 4 * g:4 * g + 4, :]
                )
            if has_bq:
                bq_sb = persist.tile([128, 1], f32, tag="bq")
                nc.sync.dma_start(out=bq_sb, in_=bq_d[:])
            if has_bk:
                bk_sb = persist.tile([128, 1], f32, tag="bk")
                nc.sync.dma_start(out=bk_sb, in_=bk_d[:])
            if has_bo:
                bo_sb = persist.tile([128, D], f32, tag="bo")
                nc.sync.dma_start(out=bo_sb, in_=bo_d[:])
            if has_gamma:
                gamma_sb = persist.tile([128, D], f32, tag="gamma")
                nc.sync.dma_start(out=gamma_sb, in_=gamma_d[:])
            if has_beta:
                beta_sb = persist.tile([128, D], f32, tag="beta")
                nc.sync.dma_start(out=beta_sb, in_=beta_d[:])

            with (
                tc.tile_pool(name="qk_ps", bufs=1, space="PSUM") as qk_psp,
                tc.tile_pool(name="xwo_ps", bufs=1, space="PSUM") as xwo_psp,
                tc.tile_pool(name="s_ps", bufs=1, space="PSUM") as s_psp,
                tc.tile_pool(name="a720_ps", bufs=1, space="PSUM") as a720_psp,
                tc.tile_pool(name="acc_ps", bufs=1, space="PSUM") as acc_psp,
                tc.tile_pool(name="z_ps", bufs=1, space="PSUM") as z_psp,
                tc.tile_pool(name="work", bufs=2) as work,
                tc.tile_pool(name="small", bufs=3) as small,
            ):
                def p0_quarter(tq):
                    t0 = tq * 512
                    for w_sb, dst, bias in (
                        (wq_sb, qT_sb, bq_sb if has_bq else None),
                        (wk_sb, kT_sb, bk_sb if has_bk else None),
                    ):
                        pr = qk_psp.tile([128, 512], f32, tag="qk")
                        for cp in range(2):
                            wap = w_sb[:, :, :]
                            xap = xT_sb[:, :, :]
                            nc.tensor.matmul(
                                out=pr,
                                lhsT=AP(tensor=wap.tensor,
                                        offset=wap.offset + 2 * cp * DK,
                                        ap=[wap.ap[0], [DK, 2], [1, DK]]),
                                rhs=AP(tensor=xap.tensor,
                                       offset=xap.offset + 2 * cp * T + t0,
                                       ap=[xap.ap[0], [T, 2], [1, 512]]),
                                start=(cp == 0), stop=(cp == 1),
                                perf_mode=PM.DoubleRow,
                            )
                        if bias is not None:
                            nc.scalar.activation(
                                out=dst[:, t0:t0 + 512], in_=pr,
                                func=AF.Identity, bias=bias[:, :], scale=1.0,
                            )
                        else:
                            nc.scalar.activation(
                                out=dst[:, t0:t0 + 512], in_=pr, func=AF.Copy
                            )
                    for tl in range(4):
                        ti = tq * 4 + tl
                        xw = xwo_psp.tile([128, 512], f32, tag="xwo")
                        for cp in range(2):
                            xap = xT_sb[:, :, :]
                            wap = wo_sb[:, :, :]
                            nc.tensor.matmul(
                                out=xw,
                                lhsT=AP(tensor=xap.tensor,
                                        offset=xap.offset + 2 * cp * T + ti * 128,
                                        ap=[xap.ap[0], [T, 2], [1, 128]]),
                                rhs=AP(tensor=wap.tensor,
                                       offset=wap.offset + 2 * cp * D,
                                       ap=[wap.ap[0], [D, 2], [1, D]]),
                                start=(cp == 0), stop=(cp == 1),
                                perf_mode=PM.DoubleRow,
                            )
                        if tl % 2 == 0:
                            nc.scalar.activation(
                                out=xWo_sb[:, ti, :], in_=xw, func=AF.Copy
                            )
                        else:
                            nc.vector.tensor_copy(out=xWo_sb[:, ti, :], in_=xw)

                def p1_block(i):
                    par = i % 2
                    w = i - 4 + par          # window start block (even)
                    jlo, jhi = max(0, i - 3), min(NBLK - 1, i + 3)
                    m0 = i - 2 + par         # mid-window start (even)

                    # ---- seeds + scores into s psum, strip slots only
                    s_ps = s_psp.tile([128, 1024], f32, tag="s")
                    slo, shi = jlo - w, jhi - w
                    # seed instrs per bank-aligned chunk covering [slo, shi]
                    for h in range(2):
                        c0, c1 = max(slo, 4 * h), min(shi + 1, 4 * h + 4)
                        if c0 >= c1:
                            continue
                        o0 = (w + c0) - i + 4
                        nc.tensor.matmul(
                            out=s_ps[:, 128 * c0:128 * c1],
                            lhsT=ident_sb[:, :],
                            rhs=M_sb[:, o0:o0 + (c1 - c0), :],
                            start=True, stop=False, skip_group_check=True,
                        )
                    for j in range(jlo, jhi + 1):
                        slot = j - w
                        nc.tensor.matmul(
                            out=s_ps[:, 128 * slot:128 * (slot + 1)],
                            lhsT=kT_sb[:, 128 * j:128 * (j + 1)],
                            rhs=qT_sb[:, 128 * i:128 * (i + 1)],
                            start=False, stop=(j == jhi),
                            skip_group_check=True,
                        )

                    # ---- exp over strip slots -> parity ET tile
                    et = etp[par]
                    etf = et[:, :, :].rearrange("p o c -> p (o c)")
                    nc.scalar.activation(
                        out=etf[:, 128 * slo:128 * (shi + 1)],
                        in_=s_ps[:, 128 * slo:128 * (shi + 1)], func=AF.Exp,
                    )
                    # edge blocks: zero pads beyond the interior fixed slot
                    fixed = 0 if par == 0 else 7
                    for sl in range(8):
                        if slo <= sl <= shi or sl == fixed:
                            continue
                        nc.gpsimd.memset(et[:, sl, :], 0.0)

                    # ---- PV720 early (only needs ET)
                    acc720 = a720_psp.tile([128, 512], f32, tag="a720")
                    acc168 = acc_psp.tile([128, 512], f32, tag="a168")
                    acc24 = acc_psp.tile([128, 512], f32, tag="a24")
                    xwoap = xWo_sb[:, :, :]

                    def pv(accp, src, base_block, npairs, close=True):
                        sap = src[:, :] if len(src.shape) == 2 else src[:, :, :]
                        pairs = [
                            k for k in range(npairs)
                            if 0 <= base_block + 2 * k <= NBLK - 2
                        ]
                        for n, k in enumerate(pairs):
                            nc.tensor.matmul(
                                out=accp,
                                lhsT=AP(tensor=sap.tensor,
                                        offset=sap.offset + 2 * k * 128,
                                        ap=[sap.ap[0], [128, 2], [1, 128]]),
                                rhs=AP(tensor=xwoap.tensor,
                                       offset=xwoap.offset
                                       + (base_block + 2 * k) * D,
                                       ap=[xwoap.ap[0], [D, 2], [1, D]]),
                                start=(n == 0),
                                stop=(close and n == len(pairs) - 1),
                                perf_mode=PM.DoubleRow,
                                skip_group_check=True,
                            )
                    pv(acc720, et, w, 4)

                    # ---- e168/e24: narrowed band spans into parity tiles
                    e168 = e168p[par]
                    e24 = e24p[par]
                    base = 128 if par == 0 else 0
                    et_mid = et[:, 2:6, :].rearrange("p o c -> p (o c)")
                    a, b = base, base + 384
                    nc.vector.tensor_mul(
                        out=e168[:, a:b], in0=et_mid[:, a:b],
                        in1=m168_sb[:, :],
                    )
                    nc.gpsimd.tensor_mul(
                        out=e24[:, a:b], in0=et_mid[:, a:b],
                        in1=m24_sb[:, :],
                    )

                    pv(acc168, e168, m0, 2)
                    pv(acc24, e24, m0, 2)

                    # ---- z sums via tiny ones-matmuls
                    zq = z_psp.tile([128, 3], f32, tag="z")
                    zc = zq[:, :]
                    onecol = ones_sb[:, 0:1]
                    etap = et[:, :, :].rearrange("p o c -> p (o c)")
                    for k in range(8):
                        nc.tensor.matmul(
                            out=zc[:, 0:1],
                            lhsT=etap[:, 128 * k:128 * (k + 1)],
                            rhs=onecol,
                            start=(k == 0), stop=(k == 7),
                        )
                    for zi, etile in ((1, e168), (2, e24)):
                        eap = etile[:, :]
                        for k in range(4):
                            nc.tensor.matmul(
                                out=zc[:, zi:zi + 1],
                                lhsT=eap[:, 128 * k:128 * (k + 1)],
                                rhs=onecol,
                                start=(k == 0), stop=(k == 3),
                            )

                    rcp = small.tile([128, 3], f32, tag="rcp")
                    nc.vector.reciprocal(out=rcp, in_=zc)

                    # ---- combine: res = x + acc720*r0 + acc168*r1 + acc24*r2
                    t2 = work.tile([128, D], f16, tag="t2")
                    nc.scalar.activation(
                        out=t2, in_=acc168, func=AF.Identity,
                        bias=0.0, scale=rcp[:, 1:2],
                    )
                    c1 = work.tile([128, D], f16, tag="c1")
                    nc.vector.scalar_tensor_tensor(
                        out=c1, in0=acc720, scalar=rcp[:, 0:1],
                        in1=x_sb[:, i, :], op0=OP.mult, op1=OP.add,
                    )
                    tt2 = work.tile([128, D], f16, tag="tt2")
                    nc.vector.scalar_tensor_tensor(
                        out=tt2, in0=acc24, scalar=rcp[:, 2:3],
                        in1=t2, op0=OP.mult, op1=OP.add,
                    )
                    nc.vector.scalar_tensor_tensor(
                        out=res_sb[:, i, :], in0=c1, scalar=1.0, in1=tt2,
                        op0=OP.mult, op1=OP.add,
                        accum_out=rsum16[:, i:i + 1],
                    )
                    if has_bo:
                        nc.gpsimd.tensor_add(
                            out=res_sb[:, i, :], in0=res_sb[:, i, :], in1=bo_sb
                        )
                    # sum of squares for LN variance
                    sqscr = work.tile([128, D], f16, tag="sqscr")
                    nc.scalar.activation(
                        out=sqscr, in_=res_sb[:, i, :], func=AF.Square,
                        accum_out=sqsum16[:, i:i + 1],
                    )

                def ln_tail(h0, hn):
                    hsl = slice(h0, h0 + hn)
                    mu = small.tile([128, hn], f32, tag="mu")
                    var = small.tile([128, hn], f32, tag="var")
                    nc.vector.tensor_scalar_mul(
                        out=mu, in0=rsum16[:, hsl], scalar1=1.0 / D
                    )
                    nc.vector.tensor_scalar_mul(
                        out=var, in0=sqsum16[:, hsl], scalar1=1.0 / D
                    )
                    musq = small.tile([128, hn], f32, tag="musq")
                    nc.vector.tensor_mul(out=musq, in0=mu, in1=mu)
                    nc.vector.tensor_sub(out=var, in0=var, in1=musq)
                    # rstd = rsqrt(var + eps): bit-hack + 2 Newton steps (DVE)
                    nc.vector.tensor_scalar_add(out=var, in0=var, scalar1=EPS)
                    rstd = small.tile([128, hn], f32, tag="rstd")
                    i32 = mybir.dt.int32
                    sh = small.tile([128, hn], i32, tag="sh")
                    nc.vector.tensor_scalar(
                        out=sh, in0=var[:, :].bitcast(i32), scalar1=1,
                        scalar2=None, op0=OP.logical_shift_right,
                    )
                    nc.vector.tensor_scalar(
                        out=rstd[:, :].bitcast(i32), in0=sh, scalar1=-1,
                        scalar2=0x5F3759DF, op0=OP.mult, op1=OP.add,
                    )
                    nt = small.tile([128, hn], f32, tag="nt")
                    for _ in range(2):
                        nc.vector.tensor_mul(out=nt, in0=rstd, in1=rstd)
                        nc.vector.tensor_mul(out=nt, in0=nt, in1=var)
                        nc.vector.tensor_scalar(
                            out=nt, in0=nt, scalar1=-0.5, scalar2=1.5,
                            op0=OP.mult, op1=OP.add,
                        )
                        nc.vector.tensor_mul(out=rstd, in0=rstd, in1=nt)
                    nmb = small.tile([128, hn], f32, tag="nmb")
                    nc.vector.tensor_mul(out=nmb, in0=mu, in1=rstd)
                    nc.vector.tensor_scalar_mul(out=nmb, in0=nmb, scalar1=-1.0)
                    for k in range(hn):
                        ib = h0 + k
                        nc.vector.tensor_scalar(
                            out=res_sb[:, ib, :], in0=res_sb[:, ib, :],
                            scalar1=rstd[:, k:k + 1], scalar2=nmb[:, k:k + 1],
                            op0=OP.mult, op1=OP.add,
                        )
                        if has_gamma:
                            nc.gpsimd.tensor_mul(
                                out=res_sb[:, ib, :], in0=res_sb[:, ib, :],
                                in1=gamma_sb,
                            )
                        if has_beta:
                            nc.gpsimd.tensor_add(
                                out=res_sb[:, ib, :], in0=res_sb[:, ib, :],
                                in1=beta_sb,
                            )
                    out_r = out_d[:].rearrange("(n p) d -> p n d", p=128)
                    for a in range(h0, h0 + hn, 2):
                        n2 = min(2, h0 + hn - a)
                        eng = [nc.sync, nc.scalar][(a // 2) % 2]
                        eng.dma_start(
                            out=out_r[:, a:a + n2, :], in_=res_sb[:, a:a + n2, :]
                        )

                # interleave: block i needs qT_i, kT strip, xWo up to i+4
                ln_after = {5: (0, 4), 9: (4, 4), 13: (8, 4), 15: (12, 2)}

                done = 0
                for tq in range(4):
                    p0_quarter(tq)
                    while done < NBLK and (
                        min(done + 3 + (done % 2), NBLK - 1)
                    ) // 4 <= tq:
                        p1_block(done)
                        if done in ln_after:
                            ln_tail(*ln_after[done])
                        done += 1
                while done < NBLK:
                    p1_block(done)
                    if done in ln_after:
                        ln_tail(*ln_after[done])
                    done += 1
                ln_tail(14, 2)

    nc.compile()
    return nc


def _get_built(flags):
    if flags not in _CACHE:
        _CACHE[flags] = _build_nc(*flags)
    return _CACHE[flags]


def _make_in_maps(x, Wq, bq, Wk, bk, Wo, bo, gamma, beta, flags):
    has_bq, has_bk, has_bo, has_gamma, has_beta = flags
    M, m168c, m24c, ident = _host_consts()
    scale = 1.0 / math.sqrt(DK)
    base = {
        "Wq8": np.ascontiguousarray(Wq * scale).astype(FP8),
        "Wk8": np.ascontiguousarray(Wk).astype(FP8),
        "Wo8": np.ascontiguousarray(Wo / 3.0).astype(FP8),
        "Mseed": M.reshape(128, 9 * 128).astype(FP8),
        "m168c": m168c.reshape(128, 384).astype(FP8),
        "m24c": m24c.reshape(128, 384).astype(FP8),
        "identb": ident.astype(FP8),
    }
    if has_bq:
        base["bq_s"] = np.ascontiguousarray(bq * scale, dtype=np.float32).reshape(DK, 1)
    if has_bk:
        base["bk_c"] = np.ascontiguousarray(bk, dtype=np.float32).reshape(DK, 1)
    if has_bo:
        base["bo_bc"] = np.broadcast_to(
            np.asarray(bo, dtype=np.float32) / 3.0, (128, D)
        ).copy()
    if has_gamma:
        base["gamma_bc"] = np.broadcast_to(
            np.asarray(gamma, dtype=np.float32), (128, D)
        ).copy()
    if has_beta:
        base["beta_bc"] = np.broadcast_to(
            np.asarray(beta, dtype=np.float32), (128, D)
        ).copy()
    maps = []
    for core in range(B):
        xc = np.ascontiguousarray(x[core], dtype=np.float32)
        maps.append({
            **base,
            "x16": xc.astype(np.float16),
            "xT8": np.ascontiguousarray(xc.T).astype(FP8),
        })
    return maps


def kernel(x, Wq, bq, Wk, bk, Wo, bo, gamma, beta):
    from concourse.bass_utils import run_bass_kernel_spmd

    x = np.asarray(x, dtype=np.float32)
    Wq = np.asarray(Wq, dtype=np.float32)
    bq = np.asarray(bq, dtype=np.float32)
    Wk = np.asarray(Wk, dtype=np.float32)
    bk = np.asarray(bk, dtype=np.float32)
    Wo = np.asarray(Wo, dtype=np.float32)
    bo = np.asarray(bo, dtype=np.float32)
    gamma = np.asarray(gamma, dtype=np.float32)
    beta = np.asarray(beta, dtype=np.float32)

    flags = (
        bool(np.any(bq != 0.0)),
        bool(np.any(bk != 0.0)),
        bool(np.any(bo != 0.0)),
        bool(np.any(gamma != 1.0)),
        bool(np.any(beta != 0.0)),
    )
    nc = _get_built(flags)
    in_maps = _make_in_maps(x, Wq, bq, Wk, bk, Wo, bo, gamma, beta, flags)
    res = run_bass_kernel_spmd(nc, in_maps, list(range(B)))
    return np.stack(
        [res.results[c]["out"].astype(np.float32) for c in range(B)], axis=0
    )
